# revision 18
# baseline (speedup 1.0000x reference)
"""MoE layer (E=8 experts, top-2 routing) on 8 Trainium2 NeuronCores.

Strategy (expert-parallel + hybrid precision):
  - The gate (T x D @ D x E, softmax, top-2, renorm) is computed on the host
    in fp32; it is ~0.01% of the FLOPs.
  - Tokens are dispatched by expert id (all-to-all done host-side): core e
    receives the tokens routed to expert e, sorted by combine weight
    descending, padded to capacity C = C_BF + C_F8.
  - Hybrid precision: the error a slot contributes to the final output is
    proportional to its combine weight, so the C_BF highest-weight slots run
    the FFN in fp16 (e10m11 inside the PE, ~8x less error than bf16 at the
    same speed) while the C_F8 lowest-weight slots run it in fp8-e4m3 via
    double-pumped DoubleRow matmuls (2x PE throughput, measured 2.0x).
    Measured fp8-FFN rel err ~5.6e-2 lands on the low-weight slots; the
    ~128 lowest-weight slots per expert run exact fp32 on the host
    (capacity 0.9375), and K_MIX fo-chunks of the lowest fp16 block's H
    run in fp8 too.  err^2 =~ 30.9e-4*(mass_fp8 + 0.52*mass_Honly).
  - The fp8 copies of w1 are derived ON DEVICE: w1 is shipped once as
    fp16(32*w1) and a [P,KO,P] staging tile per fo is cast fp16->e4m3 by
    the vector engine (0.69us/fo) just ahead of the consuming DoubleRow
    chain (0.87us/fo), double-buffered.  This removes 4.5MB of HBM traffic
    and 12 dma_starts.  w2's fp8 copy (e4m3(64*w2)) still comes by DMA into
    the fp16 w2 slot (tag reuse => WAR dep), since both fp16 source and fp8
    copy don't fit SBUF together.  All H activations use scale=1/32.
  - Big non-critical input DMAs (w2 8MB, biases) are held back with tiny
    vector-write WAW dependencies on early H tiles so the first x block and
    first w1 tiles own the HBM bandwidth at startup.
  - y DMAs are paired (two 128-token tiles per dma_start) except the last
    two, and the very last token-block's H@w2 runs as four 256-column
    chains so the final epilogue+store tail is short.  The exit drain waits
    ~115ns per dma_start semaphore (29 total).
  - Host "unshard" is two gathers + an add (each token has exactly 2 slots).
"""

import sys
import types

import numpy as np
import ml_dtypes

import concourse.bass as bass
import concourse.mybir as mybir
from concourse import bacc
from concourse.tile import TileContext
from concourse.bass_utils import run_bass_kernel_spmd


def _ensure_antenv_hooks():
    """bass_utils imports antenv.axon_hooks when BASS_TRACE is set; this image
    may lack it. Provide the registry (with the real ctypes NTFF hook when
    available) so tracing works instead of crashing."""
    try:
        import antenv.axon_hooks  # noqa: F401
        return
    except ImportError:
        pass
    if "antenv" not in sys.modules:
        try:
            import antenv  # noqa: F401
        except ImportError:
            sys.modules["antenv"] = types.ModuleType("antenv")
    hooks = types.ModuleType("antenv.axon_hooks")
    state = {"hook": None}
    hooks.set_axon_ntff_profile_hook = lambda h: state.__setitem__("hook", h)
    hooks.get_axon_ntff_profile_hook = lambda: state["hook"]
    sys.modules["antenv"].axon_hooks = hooks
    sys.modules["antenv.axon_hooks"] = hooks
    try:
        from trn_agent_boot.trn_boot import _ntff_profile_via_ctypes
        hook = _ntff_profile_via_ctypes("/opt/axon/libaxon_pjrt.so")
        if hook is not None:
            hooks.set_axon_ntff_profile_hook(hook)
    except Exception:
        pass


_ensure_antenv_hooks()

P = 128
D = 1024
F = 4096
E = 8
TOPK = 2
NBLK = 512

C_BF = 1408   # fp16 slots per core (highest combine weights)
C_F8 = 512    # fp8 slots per core (lowest combine weights); the ~128
              # lowest-weight slots per expert run exact fp32 on the host
B_SC = 32.0   # host scale on w1 before fp16 ship / device e4m3 cast
D_SC = 64.0   # host scale on w2 before e4m3 cast (w2 ~ N(0, 1/64^2))
K_MIX = 5     # fo-chunks of block-2's H computed in fp8 DoubleRow (block 2 =
              # lowest-weight fp16 slots, ranks [1024,1408), sw^2 mass 0.145;
              # each chunk adds ~0.52*30.9e-4*0.145/32 = 0.73e-5 to err^2)
NWARM = 16    # PE warm-up matmuls (HAM clock-gate ramp) during startup DMAs

# w1 fo-tile grouping: first six tiles single (block-0's H consumes one fo
# per 1.7us from ~13us in; singles stagger arrivals ahead of that pace),
# then groups of ~5 to cut DMA/semaphore count.
W1_GROUPS = ([[f] for f in range(6)]
             + [[6, 7, 8, 9, 10], [11, 12, 13, 14, 15], [16, 17, 18, 19, 20],
                [21, 22, 23, 24, 25], [26, 27, 28, 29, 30, 31]])

_F16 = np.float16
_F8 = ml_dtypes.float8_e4m3  # IEEE-style e4m3 (max 240) == TRN FP8_EXP4

_nc_cache: dict = {}
LAST = None  # BassKernelResults of the most recent run (for test harness)


def _build_moe_core(C_bf: int, C_f8: int) -> bass.Bass:
    """One-core SPMD program: fp16 FFN for C_bf tokens + fp8 FFN for C_f8."""
    dt = mybir.dt
    nc = bacc.Bacc("TRN2", target_bir_lowering=False, debug=False)
    C = C_bf + C_f8
    KO = D // P    # 8 contraction chunks for x @ w1
    FO = F // P    # 32 contraction chunks for h @ w2
    DN = D // NBLK  # 2 output-column blocks of w2
    GELU = mybir.ActivationFunctionType.Gelu
    DR = mybir.MatmulPerfMode.DoubleRow

    xt = nc.dram_tensor("xt", [D, C_bf], dt.float16, kind="ExternalInput")
    xt8 = nc.dram_tensor("xt8", [D, C_f8], dt.float8e4, kind="ExternalInput")
    # w1 host-pretiled per-fo: w1t[fo, p, ko, j] = 32*w1[ko*P+p, fo*P+j];
    # loaded in W1_GROUPS chunks (contiguous 2KB per partition per fo).
    w1t = nc.dram_tensor("w1t", [FO, P, KO, P], dt.float16,
                         kind="ExternalInput")
    w2 = nc.dram_tensor("w2", [F, D], dt.float16, kind="ExternalInput")
    w28 = nc.dram_tensor("w28", [F, D], dt.float8e4, kind="ExternalInput")
    # biases/scales: bpka = b1 partition-major [P, FO] (needed first, tiny);
    # bpkb = [b2 broadcast | b2*D_SC broadcast | sc] (held back).  The sc
    # columns for the fp8 region are pre-divided by D_SC.
    NPB = D + D + C // P
    bpka = nc.dram_tensor("bpka", [P, FO], dt.float32, kind="ExternalInput")
    bpkb = nc.dram_tensor("bpkb", [P, NPB], dt.float32, kind="ExternalInput")
    y = nc.dram_tensor("y", [C, D], dt.float16, kind="ExternalOutput")

    bf_blocks = [(off, min(NBLK, C_bf - off)) for off in range(0, C_bf, NBLK)]
    f8_blocks = [(off, min(NBLK, C - off)) for off in range(C_bf, C, NBLK)]

    xt_r = xt.rearrange("(ko p) c -> p ko c", p=P)
    xt8_r = xt8.rearrange("(ko p) c -> p ko c", p=P)

    with TileContext(nc) as tc:
        with (
            tc.tile_pool(name="w", bufs=1) as wpool,
            tc.tile_pool(name="xin", bufs=2) as xpool,
            tc.tile_pool(name="st", bufs=2) as spool,
            tc.tile_pool(name="h", bufs=1) as hpool,
            tc.tile_pool(name="yout", bufs=2) as ypool,
            tc.tile_pool(name="ph", bufs=3, space="PSUM") as phpool,
            tc.tile_pool(name="py", bufs=4, space="PSUM") as pypool,
            tc.tile_pool(name="pw", bufs=1, space="PSUM") as pwpool,
        ):
            KH = KO // 2

            def load_x_block(src_r, n_off, n_size, f8, split, dep=None):
                ddt = dt.float8e4 if f8 else dt.float16
                xf = xpool.tile([P, KO, NBLK], ddt, tag="xf")
                if dep is not None:  # WAW stub: keep this load off the
                    nc.vector.tensor_scalar_mul(  # startup DMA window
                        xf[:1, 0, :1], dep, 0.0)
                if split:  # block 0: four DMAs so the first H chain's
                    for kq in range(0, KO, 2):  # ko chunks land in 256KB
                        nc.sync.dma_start(      # steps instead of one 1MB
                            xf[:, kq:kq + 2, :n_size],
                            src_r[:, kq:kq + 2, n_off:n_off + n_size])
                else:
                    nc.sync.dma_start(
                        xf[:, :, :n_size],
                        src_r[:, :, n_off:n_off + n_size])
                return xf

            # Warm the PE's HAM clock gate during the startup DMA window
            # with dummy matmuls on zeroed SBUF.
            warm = wpool.tile([P, NBLK], dt.float16, tag="warm")
            nc.gpsimd.memset(warm[:], 0.0)
            pwarm = pwpool.tile([P, NBLK], dt.float32, tag="pw")
            for i in range(NWARM):
                nc.tensor.matmul(
                    pwarm[:], warm[:, :P], warm[:],
                    start=(i == 0), stop=(i == NWARM - 1),
                )

            xf0 = load_x_block(xt_r, *bf_blocks[0], False, True)

            b1sb = wpool.tile([P, FO], dt.float32, tag="bpka")
            nc.sync.dma_start(b1sb[:], bpka[:])

            w1sb = [None] * FO

            def load_w1_group(gi, dep=None):
                grp = W1_GROUPS[gi]
                t_ = wpool.tile([P, len(grp), KO, P], dt.float16,
                                tag=f"w1g_{gi}", name=f"w1g{gi}")
                if dep is not None:  # WAW stub: hold the load off the
                    nc.vector.tensor_scalar_mul(  # startup DMA window
                        t_[:1, 0, 0, :1], dep, 0.0)
                nc.sync.dma_start(
                    t_[:],
                    w1t[grp[0]:grp[-1] + 1].rearrange("f p k j -> p f k j"),
                )
                for i, fo in enumerate(grp):
                    w1sb[fo] = t_[:, i]

            for gi in range(len(W1_GROUPS)):  # w1 + x own the startup window
                load_w1_group(gi)             # (w2/biases are held back)

            late: dict = {}

            def stage_w1_f8(fo):
                """Vector-cast one fo of fp16 w1 (pre-scaled x32) to e4m3."""
                st = spool.tile([P, KO, P], dt.float8e4, tag="w1s")
                nc.vector.tensor_scalar_mul(st[:], w1sb[fo], 1.0)
                return st

            def y_group_epilogue(yt, li, py, bias_sb, tbg, dsl):
                nc.vector.tensor_add(yt[:, li, dsl], py[:], bias_sb[:, dsl])
                nc.vector.tensor_scalar_mul(
                    yt[:, li, dsl], yt[:, li, dsl], late["scsb"][:, tbg:tbg + 1]
                )

            def y_groups(n_size, last_block):
                tbs = n_size // P
                groups = [(t, min(2, tbs - t)) for t in range(0, tbs, 2)]
                if last_block and groups and groups[-1][1] == 2:
                    t0, _ = groups.pop()
                    groups += [(t0, 1), (t0 + 1, 1)]
                return groups

            # ---------------- pass 1: fp16 blocks ----------------
            prev_h = None
            for bi, (n_off, n_size) in enumerate(bf_blocks):
                xf = (xf0 if bi == 0
                      else load_x_block(xt_r, n_off, n_size, False, False,
                                        dep=(prev_h[:1, 5, :1]
                                             if bi == 1 else None)))
                if bi == 2:
                    # fp8 x for the K_MIX chunks, cast on device from fp16
                    xf8b = xpool.tile([P, KO, NBLK], dt.float8e4,
                                      tag="xf8b", bufs=1)
                    nc.vector.tensor_scalar_mul(
                        xf8b[:, :, :n_size], xf[:, :, :n_size], 1.0)

                htile = hpool.tile([P, FO, NBLK], dt.float16, tag="h")
                # block 2 runs its bf16 fo's first: the vector engine casts
                # xf8b + the fp8 w1 stagings during that window, so the
                # trailing K_MIX DoubleRow chunks start with zero bubble
                fo_order = (list(range(K_MIX, FO)) + list(range(K_MIX))
                            if bi == 2 else range(FO))
                for fo in fo_order:
                    ph = phpool.tile([P, NBLK], dt.float32, tag="ph")
                    if bi == 2 and fo < K_MIX:  # fp8 DoubleRow chunk
                        st = stage_w1_f8(fo)
                        for j in range(KO // 2):
                            nc.tensor.matmul(
                                ph[:, :n_size],
                                st[:, 2 * j:2 * j + 2, :],
                                xf8b[:, 2 * j:2 * j + 2, :n_size],
                                start=(j == 0),
                                stop=(j == KO // 2 - 1),
                                perf_mode=DR,
                            )
                    else:
                        for ko in range(KO):
                            nc.tensor.matmul(
                                ph[:, :n_size],
                                w1sb[fo][:, ko, :],
                                xf[:, ko, :n_size],
                                start=(ko == 0),
                                stop=(ko == KO - 1),
                            )
                    nc.scalar.activation(
                        htile[:, fo, :n_size], ph[:, :n_size], GELU,
                        bias=b1sb[:, fo:fo + 1], scale=1.0 / B_SC,
                    )
                    if bi == 0 and fo == 12:
                        # release the 8MB w2 load only now (needed at ~68us):
                        # a 1-elem vector write WAW dep keeps it out of the
                        # startup window where x block 0 + w1 must land
                        w2sb = wpool.tile([P, FO, D], dt.float16, tag="w2")
                        nc.vector.tensor_scalar_mul(
                            w2sb[:1, 0, :1], htile[:1, 12, :1], 0.0)
                        nc.sync.dma_start(
                            w2sb[:], w2.rearrange("(fo p) d -> p fo d", p=P))
                        late["w2sb"] = w2sb
                    if bi == 0 and fo == 3:
                        bsb2 = wpool.tile([P, NPB], dt.float32, tag="bpkb")
                        nc.vector.tensor_scalar_mul(
                            bsb2[:1, :1], htile[:1, 3, :1], 0.0)
                        nc.sync.dma_start(bsb2[:], bpkb[:])
                        late["b2sb"] = bsb2[:, :D]
                        late["b2sb8"] = bsb2[:, D:2 * D]
                        late["scsb"] = bsb2[:, 2 * D:]

                prev_h = htile
                for t0, cnt in y_groups(n_size, False):
                    yt = ypool.tile([P, 2, D], dt.float16, tag="y")
                    for tb in range(t0, t0 + cnt):
                        tbg = (n_off + tb * P) // P
                        # dn-outer: the d-half 0 epilogue overlaps d-half 1
                        # matmuls.
                        for dn in range(DN):
                            dsl = slice(dn * NBLK, (dn + 1) * NBLK)
                            py = pypool.tile([P, NBLK], dt.float32, tag="py")
                            for fo in range(FO):
                                nc.tensor.matmul(
                                    py[:],
                                    htile[:, fo, tb * P:(tb + 1) * P],
                                    late["w2sb"][:, fo, dsl],
                                    start=(fo == 0),
                                    stop=(fo == FO - 1),
                                )
                            y_group_epilogue(yt, tb - t0, py,
                                             late["b2sb"], tbg, dsl)
                    g0 = (n_off + t0 * P) // P
                    nc.sync.dma_start(
                        y[g0 * P:(g0 + cnt) * P, :].rearrange(
                            "(t p) d -> p t d", p=P),
                        yt[:, :cnt],
                    )

            # ---------------- pass 2: fp8 DoubleRow blocks ----------------
            # e4m3(64*w2) reuses the fp16 w2 slot (same pool tag => WAR dep
            # on the last pass-1 reader, so the swap overlaps pass-1 compute)
            w2sb8 = wpool.tile([P, FO, D], dt.float8e4, tag="w2")
            nc.sync.dma_start(
                w2sb8[:], w28.rearrange("(fo p) d -> p fo d", p=P)
            )

            for blki, (n_off, n_size) in enumerate(f8_blocks):
                xf = load_x_block(xt8_r, n_off - C_bf, n_size, True, False)

                htile8 = hpool.tile([P, FO, NBLK], dt.float8e4, tag="h")
                for fo in range(FO):
                    st = stage_w1_f8(fo)
                    ph = phpool.tile([P, NBLK], dt.float32, tag="ph")
                    for j in range(KO // 2):
                        nc.tensor.matmul(
                            ph[:, :n_size],
                            st[:, 2 * j:2 * j + 2, :],
                            xf[:, 2 * j:2 * j + 2, :n_size],
                            start=(j == 0),
                            stop=(j == KO // 2 - 1),
                            perf_mode=DR,
                        )
                    # ph = B_SC*(x@w1); H8 = gelu(ph/B_SC + b1)
                    nc.scalar.activation(
                        htile8[:, fo, :n_size], ph[:, :n_size], GELU,
                        bias=b1sb[:, fo:fo + 1], scale=1.0 / B_SC,
                    )

                last_block = blki == len(f8_blocks) - 1
                groups = y_groups(n_size, last_block)
                for gi, (t0, cnt) in enumerate(groups):
                    yt = ypool.tile([P, 2, D], dt.float16, tag="y")
                    final = last_block and gi == len(groups) - 1
                    for tb in range(t0, t0 + cnt):
                        tbg = (n_off + tb * P) // P
                        # the very last token-block runs in four 256-column
                        # chains so the closing epilogue+store is short
                        nq = 4 if final else DN
                        qw = D // nq
                        for q in range(nq):
                            dsl = slice(q * qw, (q + 1) * qw)
                            py = pypool.tile([P, qw], dt.float32, tag="py")
                            for j in range(FO // 2):
                                nc.tensor.matmul(
                                    py[:],
                                    htile8[:, 2 * j:2 * j + 2,
                                           tb * P:(tb + 1) * P],
                                    w2sb8[:, 2 * j:2 * j + 2, dsl],
                                    start=(j == 0),
                                    stop=(j == FO // 2 - 1),
                                    perf_mode=DR,
                                )
                            # py = D_SC*(H@w2); y = (py + D_SC*b2)*(sc/D_SC)
                            y_group_epilogue(yt, tb - t0, py,
                                             late["b2sb8"], tbg, dsl)
                            if final:  # store each 64KB quarter as it
                                nc.sync.dma_start(  # finishes: short tail
                                    y[tbg * P:(tbg + 1) * P, dsl],
                                    yt[:, 0, dsl],
                                )
                    if not final:
                        g0 = (n_off + t0 * P) // P
                        nc.sync.dma_start(
                            y[g0 * P:(g0 + cnt) * P, :].rearrange(
                                "(t p) d -> p t d", p=P),
                            yt[:, :cnt],
                        )
    nc.compile()
    return nc


def _route(flat, gate_w, gate_b):
    """fp32 gate matching the reference: softmax, top-2, renormalize."""
    logits = flat @ gate_w + gate_b
    m = logits.max(axis=1, keepdims=True)
    p = np.exp(logits - m, dtype=np.float32)
    probs = p / p.sum(axis=1, keepdims=True)
    ti = np.argsort(-probs, axis=1, kind="stable")[:, :TOPK]
    tp = np.take_along_axis(probs, ti, axis=1)
    sw = tp / (tp.sum(axis=1, keepdims=True) + np.float32(1e-9))
    return ti.astype(np.int64), sw.astype(np.float32)


def _gelu_exact(v):
    try:
        from scipy.special import erf
        return 0.5 * v * (1.0 + erf(v / np.sqrt(2.0)))
    except ImportError:  # tanh approximation fallback (overflow tokens only)
        return 0.5 * v * (1.0 + np.tanh(
            0.7978845608028654 * (v + 0.044715 * v ** 3)))


def kernel(**inputs) -> np.ndarray:
    global LAST
    x = np.asarray(inputs["x"], np.float32)
    gate_w = np.asarray(inputs["gate_w"], np.float32)
    gate_b = np.asarray(inputs["gate_b"], np.float32)
    w1 = np.asarray(inputs["w1"], np.float32)
    b1 = np.asarray(inputs["b1"], np.float32)
    w2 = np.asarray(inputs["w2"], np.float32)
    b2 = np.asarray(inputs["b2"], np.float32)

    B, S, D_ = x.shape
    flat = x.reshape(-1, D_)
    Tn = flat.shape[0]

    ti, sw = _route(flat, gate_w, gate_b)
    sw_flat = sw.ravel()
    flat_e = ti.ravel()

    C = C_BF + C_F8
    KO, FO = D // P, F // P

    xT_16 = np.ascontiguousarray(flat.T).astype(_F16)    # [D, T] fp16
    xT_f8 = np.ascontiguousarray(flat.T).astype(_F8)     # [D, T] e4m3

    # Per-expert slot ranking by combine weight (descending): ranks
    # [0, C_BF) run fp16, [C_BF, C) run fp8, the tail runs on the host.
    in_maps = []
    slot_expert = np.empty(Tn * TOPK, np.int64)
    slot_rank = np.empty(Tn * TOPK, np.int64)
    overflow = []
    for e in range(E):
        pairs = np.nonzero(flat_e == e)[0]
        pairs = pairs[np.argsort(-sw_flat[pairs], kind="stable")]
        n_e = len(pairs)
        slot_expert[pairs] = e
        slot_rank[pairs] = np.arange(n_e)
        if n_e > C:
            overflow.append((e, pairs[C:]))

        bf = pairs[:min(C_BF, n_e)]
        f8 = pairs[C_BF:min(C, n_e)]
        toks_bf = bf // TOPK
        toks_f8 = f8 // TOPK

        xt_e = np.zeros((D, C_BF), _F16)
        xt_e[:, :len(bf)] = xT_16[:, toks_bf]
        xt8_e = np.zeros((D, C_F8), _F8)
        xt8_e[:, :len(f8)] = xT_f8[:, toks_f8]

        sc_e = np.zeros((C,), np.float32)
        sc_e[:len(bf)] = sw_flat[bf]
        sc_e[C_BF:C_BF + len(f8)] = sw_flat[f8] / np.float32(D_SC)

        w1_16 = (w1[e] * B_SC).astype(_F16).reshape(KO, P, FO, P)
        bpka_e = np.ascontiguousarray(b1[e].reshape(FO, P).T).astype(
            np.float32)
        bpkb_e = np.concatenate([
            np.broadcast_to(b2[e], (P, D)),
            np.broadcast_to(b2[e] * D_SC, (P, D)),
            np.ascontiguousarray(sc_e.reshape(C // P, P).T),
        ], axis=1).astype(np.float32)
        in_maps.append({
            "xt": xt_e,
            "xt8": xt8_e,
            "w1t": np.ascontiguousarray(w1_16.transpose(2, 1, 0, 3)),
            "w2": w2[e].astype(_F16),
            "w28": (w2[e] * D_SC).astype(_F8),
            "bpka": np.ascontiguousarray(bpka_e),
            "bpkb": np.ascontiguousarray(bpkb_e),
        })

    key = (C_BF, C_F8)
    nc = _nc_cache.get(key)
    if nc is None:
        nc = _build_moe_core(C_BF, C_F8)
        _nc_cache[key] = nc

    LAST = run_bass_kernel_spmd(nc, in_maps, core_ids=list(range(E)))
    Yall = np.stack([np.asarray(LAST.results[i]["y"]).astype(np.float32)
                     for i in range(E)])

    # Combine: device slots via two gathers; host fp32 FFN for overflow.
    in_cap = slot_rank < C
    contrib = np.zeros((Tn * TOPK, D_), np.float32)
    idx = np.nonzero(in_cap)[0]
    contrib[idx] = Yall[slot_expert[idx], slot_rank[idx]]
    out = contrib[0::TOPK] + contrib[1::TOPK]
    for e, over in overflow:
        toks = over // TOPK
        h = _gelu_exact(flat[toks] @ w1[e] + b1[e])
        y_e = h @ w2[e] + b2[e]
        out[toks] += sw_flat[over][:, None] * y_e
    return out.reshape(B, S, D_).astype(np.float32)
